# revision 92
# baseline (speedup 1.0000x reference)
"""Trainium2 Bass kernel for nn_NodeBlock (GNN message passing).

Pipeline: segment_sum of edge features onto destination nodes, concat with
node features, 3-layer MLP, LayerNorm.

Sharding: nodes are range-sharded across the 8 cores (12800 nodes/core, 100
blocks of 128). On the host, edges are bucketed by destination-node block.
Within a block, the first T edges of each node are packed "dense":
feature-major tiles dT[f, t*128 + j] = (t-th edge of node j)[f], so the
on-device aggregation for them is a plain PSUM accumulation via an
identity-weight matmul (no one-hot needed). Edges beyond T per node go to
"overflow" tiles in slot-major layout with a per-tile one-hot built by a
DVE is_equal against an iota row (like a classic scatter-add matmul).

All edge/node/weight data is fp16 (rel err ~3e-4, far inside the 2e-2
budget), so the PE runs at 1 cycle/row and edge HBM traffic is 2B/elem.

MLP runs feature-major, batched over groups of 4 blocks (free dim 512).
LayerNorm: W2 is column-centered on the host so the mean term vanishes;
variance comes from an ACT Square+accum_out on the transposed block;
rstd is folded into the PSUM evacuation; gamma/beta are folded into one
ACT (scale/bias per-partition) after transposing back to feature-major.
Output is fp16 feature-major; the host transposes/upcasts.
"""

import sys

sys.path.insert(0, "/opt/trn_rl_repo")

import numpy as np

N_CORES = 8
NUM_NODES = 100000
D = 128            # node/edge feature dim
P = 128            # partitions
BLK = 128          # nodes per block
BLOCKS_PER_CORE = 100
NODES_PER_CORE = BLK * BLOCKS_PER_CORE   # 12800
TOTAL_BLOCKS = N_CORES * BLOCKS_PER_CORE  # 800
GRP = 4            # blocks per MLP group (free dim 512)
NGRP = BLOCKS_PER_CORE // GRP            # 25
EPS = 1e-5

_nc_cache = {}
last_run_info = {}

TUNE = {
    "ovf_lambda": 0.3,  # extra planner cost per overflow (IS_EQ) tile
    "chunk": 2,         # blocks per edge DMA
    "ebufs": 8, "ohbufs": 16, "sbufs": 4,
    "agbufs": 3, "mlpbufs": 3, "tyxbufs": 2,
    "out_sync_tail": 5,   # last N groups' out-DMAs go to the sync queue
    "prefetch_pairs": 2,  # loop mode: chunks prefetched across the barrier
    "split_evac": True,
    "out_queue": "gpsimd",      # gpsimd | dma_queue setting
    "oh_engine": "vector",      # vector | gpsimd | split
    "aggr_evac": "vector",      # vector | scalar
    "relu_engine": "vector",    # scalar | vector
    "stats_engine": "vector",   # scalar (Square+accum) | vector | split
    "yot_engine": "scalar",     # scalar | gpsimd
    "xn_engine": "vector",      # vector | scalar
    "dma_queue": "sync",        # sync | alt  (stream DMA issue queue)
}


def _build_nc(kb, loop_iters=None):
    """kb: ("v3"|"v3t", (T_b,)*100, (V_b,)*100) — per-position dense depth
    T_b and overflow tile count V_b, shared across the 8 cores.
    "v3t" = trivial LayerNorm affine (gamma==1, beta==0): skip the
    gamma/beta stage and the second transpose; output is node-major."""
    import contextlib
    import concourse.bacc as bacc
    import concourse.tile as tile
    import concourse.mybir as mybir
    from concourse.masks import make_identity

    dt = mybir.dt
    f32 = dt.float32
    f16 = dt.float16
    variant, ns, vb = kb
    trivial = variant.endswith("t")
    ns = [list(n) for n in ns]           # per-GROUP dense tile widths
    vb = list(vb)
    tot_v = sum(vb)                      # total overflow tiles per core
    grp_elems = [sum(n) + v * 128 for n, v in zip(ns, vb)]
    tot_e = sum(grp_elems)
    # two chunks per group, split at a dense-tile boundary near half
    chunk_spans = []
    for g in range(NGRP):
        half = grp_elems[g] // 2
        acc = 0
        s0 = 0
        for m in ns[g]:
            if acc + m > half and acc > 0:
                break
            acc += m
        s0 = acc
        chunk_spans.append((s0, grp_elems[g] - s0))

    nc = bacc.Bacc("TRN2", target_bir_lowering=False, debug=False,
                   name="nodeblock")

    edges = nc.dram_tensor("edges", [P, tot_e], f16, kind="ExternalInput")
    iota_in = nc.dram_tensor("iota", [P, P], f16, kind="ExternalInput")
    colv = nc.dram_tensor("colv", [P, max(tot_v, 1)], f32,
                          kind="ExternalInput")
    natT = nc.dram_tensor("natT", [P, NODES_PER_CORE], f16,
                          kind="ExternalInput")
    w_in = {}
    for nm in ["w0a", "w0b", "w1", "w2c"]:
        w_in[nm] = nc.dram_tensor(nm, [128, 128], f16, kind="ExternalInput")
    for nm in ["b0", "b1", "b2c", "gam", "bet"]:
        w_in[nm] = nc.dram_tensor(nm, [128, 1], f32, kind="ExternalInput")
    out = nc.dram_tensor("out", [NGRP, P, GRP * BLK], f16,
                         kind="ExternalOutput")

    with tile.TileContext(nc) as tc:
        with (
            tc.tile_pool(name="const", bufs=1) as cpool,
            tc.tile_pool(name="edge", bufs=TUNE["ebufs"]) as epool,
            tc.tile_pool(name="oh", bufs=TUNE["ohbufs"]) as ohpool,
            tc.tile_pool(name="small", bufs=TUNE["sbufs"]) as spool,
            tc.tile_pool(name="psag", bufs=TUNE["agbufs"],
                         space="PSUM") as psag,
            tc.tile_pool(name="psmlp", bufs=TUNE["mlpbufs"],
                         space="PSUM") as psmlp,
            tc.tile_pool(name="pstyx", bufs=TUNE["tyxbufs"],
                         space="PSUM") as pstyx,
        ):
            # small/gating constants first: iota+colv gate every IS_EQ,
            # weights gate the first MLP; the 3.3MB natT load goes last
            iota = cpool.tile([P, P], f16, tag="iota", name="iota")
            nc.scalar.dma_start(out=iota[:], in_=iota_in[:])
            colv_s = cpool.tile([P, max(tot_v, 1)], f32, tag="colv",
                                name="colv")
            nc.scalar.dma_start(out=colv_s[:], in_=colv[:])
            consts = {}
            for nm, t in w_in.items():
                cdt = f16 if nm in ("w0a", "w0b", "w1", "w2c") else f32
                consts[nm] = cpool.tile(list(t.shape), cdt, tag=nm, name=nm)
                nc.scalar.dma_start(out=consts[nm][:], in_=t[:])
            natT_s = cpool.tile([P, NODES_PER_CORE], f16, tag="natT",
                                name="natT")
            nc.scalar.dma_start(out=natT_s[:], in_=natT[:])
            ident = cpool.tile([P, P], f16, tag="ident", name="ident")
            make_identity(nc, ident[:])
            consts["iota"] = iota
            epst = cpool.tile([P, 1], f32, tag="eps", name="eps")
            nc.vector.memset(epst[:], EPS)
            consts["eps"] = epst
            ones_c = cpool.tile([P, 1], f16, tag="ones_c", name="ones_c")
            nc.vector.memset(ones_c[:], 1.0)
            consts["ones_c"] = ones_c
            ones_r = cpool.tile([1, P], f32, tag="ones_r", name="ones_r")
            nc.vector.memset(ones_r[:], 1.0)
            consts["ones_r"] = ones_r

            # prefetch tiles for the first chunks: loaded in the preamble,
            # then re-loaded at the END of each loop-body execution so the
            # next iteration's aggregation starts with data already in SBUF
            # flat chunk list: (group, span); prefetch the first npf
            flat_chunks = []
            eo = 0
            for g in range(NGRP):
                for s in chunk_spans[g]:
                    flat_chunks.append((eo, s))
                    eo += s
            npf = TUNE.get("prefetch_pairs", 0)
            pf_tiles = []
            for i in range(npf):
                eoff, span = flat_chunks[i]
                pft = cpool.tile([P, span], f16, tag=f"pf{i}",
                                 name=f"pf{i}")
                nc.sync.dma_start(out=pft[:],
                                  in_=edges[:, eoff:eoff + span])
                pf_tiles.append((pft, eoff, span))

            loop_cm = (tc.For_i(0, loop_iters, 1) if loop_iters
                       else contextlib.nullcontext())
            with loop_cm:
                _emit_blocks(nc, tc, ns, vb, chunk_spans, epool, ohpool,
                             spool, psag, psmlp, pstyx, colv_s, natT_s,
                             consts, ident, edges, out, mybir, trivial,
                             pf_tiles)
                if loop_iters:
                    for pft, eoff, span in pf_tiles:
                        nc.sync.dma_start(out=pft[:],
                                          in_=edges[:, eoff:eoff + span])
    nc.finalize()
    return nc


def _emit_blocks(nc, tc, ns, vb, chunk_spans, epool, ohpool, spool, psag,
                 psmlp, pstyx, colv_s, natT_s, consts, ident, edges, out,
                 mybir, trivial=False, pf_tiles=()):
    dt = mybir.dt
    f32 = dt.float32
    f16 = dt.float16
    Alu = mybir.AluOpType
    Act = mybir.ActivationFunctionType
    iota = consts["iota"]
    W = GRP * BLK

    grp_elems = [sum(n) + v * 128 for n, v in zip(ns, vb)]
    max_span = max(max(s) for s in chunk_spans)
    g_offs, v_offs = [], []
    eo = vo = 0
    for g in range(NGRP):
        g_offs.append(eo)
        v_offs.append(vo)
        eo += grp_elems[g]
        vo += vb[g]

    for g in range(NGRP):
        # two chunk DMAs per group (split at a dense-tile boundary)
        s0, s1 = chunk_spans[g]
        tiles_c = []
        coff = g_offs[g]
        for ci, span in enumerate((s0, s1)):
            idx = 2 * g + ci
            if idx < len(pf_tiles):
                tiles_c.append(pf_tiles[idx][0])
            else:
                ct = epool.tile([P, max_span], f16, tag="eblk",
                                name="eblk")
                nc.sync.dma_start(out=ct[:, :span],
                                  in_=edges[:, coff:coff + span])
                tiles_c.append(ct)
            coff += span

        def ebslice(off, width):
            if off + width <= s0:
                return tiles_c[0][:, off:off + width]
            return tiles_c[1][:, off - s0:off - s0 + width]

        pag = psag.tile([P, W], f32, tag="ag", name="ag")
        vec_stats = TUNE["stats_engine"] == "vector"
        ssq = (None if vec_stats else
               spool.tile([P, GRP], f32, tag="ssq", name="ssq"))
        mv = (spool.tile([P, 2 * GRP], f32, tag="mv", name="mv")
              if TUNE["stats_engine"] != "scalar" else None)
        py_tiles = []
        V = vb[g]
        # one-hots first so DVE runs ahead of the PE
        ohs = []
        for v in range(V):
            oh = ohpool.tile([P, 128], f16, tag="oh", name="oh")
            nc.vector.tensor_scalar(
                out=oh[:], in0=iota[:],
                scalar1=colv_s[:, v_offs[g] + v:v_offs[g] + v + 1],
                scalar2=None, op0=Alu.is_equal)
            ohs.append(oh)
        # single group-wide dense accumulation chain, then overflow
        N = ns[g]
        T = len(N)
        off = 0
        for t, n in enumerate(N):
            nc.tensor.matmul(out=pag[:, :n], lhsT=ident[:],
                             rhs=ebslice(off, n),
                             start=(t == 0),
                             stop=(t == T - 1 and V == 0),
                             skip_group_check=True)
            off += n
        for v in range(V):
            nc.tensor.matmul(
                out=pag[:, :BLK], lhsT=ebslice(off + v * 128, 128),
                rhs=ohs[v][:], start=False, stop=(v == V - 1),
                skip_group_check=True)

        # aggregated fp16 copy PSUM -> SBUF
        aggrT = spool.tile([P, GRP * BLK], f16, tag="aggrT", name="aggrT")
        H = GRP * BLK // 2
        if TUNE["split_evac"]:
            # two parallel half-evacs on different engines so the first
            # half of the W0b matmul can start sooner
            nc.vector.tensor_copy(aggrT[:, :H], pag[:, :H])
            nc.scalar.copy(aggrT[:, H:], pag[:, H:])
        elif TUNE["aggr_evac"] == "vector":
            nc.vector.tensor_copy(aggrT[:], pag[:])
        else:
            nc.scalar.copy(aggrT[:], pag[:])

        # MLP over the whole group (free dim 512)
        ph1 = psmlp.tile([P, GRP * BLK], f32, tag="mlp", name="mlp")
        nc.tensor.matmul(out=ph1[:], lhsT=consts["w0a"][:],
                         rhs=natT_s[:, g * GRP * BLK:(g + 1) * GRP * BLK],
                         start=True, stop=False)
        if TUNE["split_evac"]:
            nc.tensor.matmul(out=ph1[:, :H], lhsT=consts["w0b"][:],
                             rhs=aggrT[:, :H], start=False, stop=True,
                             skip_group_check=True)
            nc.tensor.matmul(out=ph1[:, H:], lhsT=consts["w0b"][:],
                             rhs=aggrT[:, H:], start=False, stop=True,
                             skip_group_check=True)
        else:
            nc.tensor.matmul(out=ph1[:], lhsT=consts["w0b"][:],
                             rhs=aggrT[:], start=False, stop=True)
        h1 = spool.tile([P, GRP * BLK], f16, tag="h1", name="h1")
        if TUNE["relu_engine"] == "scalar":
            nc.scalar.activation(h1[:], ph1[:], Act.Relu,
                                 bias=consts["b0"][:])
        else:
            nc.vector.tensor_scalar(out=h1[:], in0=ph1[:],
                                    scalar1=consts["b0"][:], scalar2=0.0,
                                    op0=Alu.add, op1=Alu.max)

        ph2 = psmlp.tile([P, GRP * BLK], f32, tag="mlp", name="mlp")
        nc.tensor.matmul(out=ph2[:], lhsT=consts["w1"][:], rhs=h1[:],
                         start=True, stop=True)
        h2 = spool.tile([P, GRP * BLK], f16, tag="h2", name="h2")
        nc.scalar.activation(h2[:], ph2[:], Act.Relu, bias=consts["b1"][:])

        ph3 = psmlp.tile([P, GRP * BLK], f32, tag="mlp", name="mlp")
        nc.tensor.matmul(out=ph3[:], lhsT=consts["w2c"][:], rhs=h2[:],
                         start=True, stop=True)
        h3T = spool.tile([P, GRP * BLK], f16, tag="h3T", name="h3T")
        nc.scalar.activation(h3T[:], ph3[:], Act.Identity,
                             bias=consts["b2c"][:])

        # per block: transpose to node-major, Square+accum for variance
        tyx_w = GRP * BLK if trivial else 2 * GRP * BLK
        tyx = pstyx.tile([P, tyx_w], f16, tag="tyx", name="tyx")
        py_g = tyx[:, :GRP * BLK]
        pxt = None if trivial else tyx[:, GRP * BLK:]
        for q in range(GRP):
            py = py_g[:, q * BLK:(q + 1) * BLK]
            nc.tensor.transpose(py, h3T[:, q * BLK:(q + 1) * BLK],
                                ident[:])
            py_tiles.append(py)
            se = TUNE["stats_engine"]
            if se == "split":
                se = "scalar" if q % 2 == 0 else "vector"
            if se == "scalar":
                sq = spool.tile([P, BLK], f16, tag="sq", name="sq")
                nc.scalar.activation(sq[:], py, Act.Square,
                                     accum_out=ssq[:, q:q + 1])
            else:
                # bn_stats/bn_aggr: mv[:, 2q:2q+2] = (mean, var); the
                # variance is already /N (sqrt uses scale=1.0 for it)
                st6 = spool.tile([P, 6], f32, tag="st6", name="st6")
                nc.vector.bn_stats(st6[:], py)
                nc.vector.bn_aggr(mv[:, 2 * q:2 * q + 2], st6[:])

        # rstd for the 4 blocks at once: 1/sqrt(var + eps)
        std = spool.tile([P, GRP], f32, tag="std", name="std")
        if TUNE["stats_engine"] == "vector":
            nc.scalar.activation(std[:], mv[:, 1::2], Act.Sqrt,
                                 bias=consts["eps"][:], scale=1.0)
        else:
            nc.scalar.activation(std[:], ssq[:], Act.Sqrt,
                                 bias=consts["eps"][:], scale=1.0 / BLK)
        rstd = spool.tile([P, GRP], f32, tag="rstd", name="rstd")
        nc.vector.reciprocal(rstd[:], std[:])

        # xn = py * rstd (node-major)
        if trivial:
            # gamma==1, beta==0: xn IS the output (node-major layout)
            xn_g = spool.tile([P, GRP * BLK], f16, tag="yoT", name="yoT")
            for q in range(GRP):
                xn = xn_g[:, q * BLK:(q + 1) * BLK]
                if TUNE["xn_engine"] == "vector":
                    nc.vector.tensor_scalar(out=xn, in0=py_tiles[q],
                                            scalar1=rstd[:, q:q + 1],
                                            scalar2=None, op0=Alu.mult)
                else:
                    nc.scalar.activation(xn, py_tiles[q], Act.Copy,
                                         scale=rstd[:, q:q + 1])
            yoT = xn_g
            if g >= NGRP - TUNE.get("out_sync_tail", 0):
                nc.sync.dma_start(out=out[g], in_=yoT[:])
                continue
        else:
            for q in range(GRP):
                xn = spool.tile([P, BLK], f16, tag="xn", name="xn")
                if TUNE["xn_engine"] == "vector":
                    nc.vector.tensor_scalar(out=xn[:], in0=py_tiles[q],
                                            scalar1=rstd[:, q:q + 1],
                                            scalar2=None, op0=Alu.mult)
                else:
                    nc.scalar.activation(xn[:], py_tiles[q], Act.Copy,
                                         scale=rstd[:, q:q + 1])
                nc.tensor.transpose(pxt[:, q * BLK:(q + 1) * BLK], xn[:],
                                    ident[:])
            yoT = spool.tile([P, GRP * BLK], f16, tag="yoT", name="yoT")
            if TUNE["yot_engine"] == "scalar":
                nc.scalar.activation(yoT[:], pxt[:], Act.Identity,
                                     bias=consts["bet"][:],
                                     scale=consts["gam"][:])
            else:
                nc.gpsimd.tensor_scalar(out=yoT[:], in0=pxt[:],
                                        scalar1=consts["gam"][:],
                                        scalar2=consts["bet"][:],
                                        op0=Alu.mult, op1=Alu.add)
        if TUNE["out_queue"] == "gpsimd":
            odma = nc.gpsimd
        elif TUNE["dma_queue"] == "sync":
            odma = nc.sync
        else:
            odma = nc.scalar if g % 2 == 0 else nc.sync
        odma.dma_start(out=out[g], in_=yoT[:])


def _plan_layout(deg_gb):
    """deg_gb: [800, 128] per-global-block node degrees.

    Assign the 800 blocks to 100 positions x 8 cores so that blocks with
    similar degree profiles share a position (minimizing the cross-core
    max padding), and pick a per-position dense depth T_b minimizing
    tiles = T_b + max_core ceil(overflow/128). With nodes sorted by
    degree within each block, dense tile t only needs
    n_t = max_core #{deg > t} columns; returns those widths too."""
    order = np.argsort(-deg_gb.sum(axis=1), kind="stable")
    assign = order.reshape(BLOCKS_PER_CORE, N_CORES)  # [pos, core] -> gb
    W = GRP * BLK
    ns = []
    vb = np.zeros(NGRP, np.int64)
    for g in range(NGRP):
        # per-core degree profile over the whole 512-node group
        dg = np.concatenate(
            [deg_gb[assign[g * GRP + q]] for q in range(GRP)],
            axis=1)                            # [8, 512]
        dmax = int(dg.max())
        cnt = np.stack([(dg > t).sum(axis=1).max(axis=0)
                        for t in range(max(dmax, 1) + 1)])  # max_c count
        # overflow nodes must fit the first 128 group columns
        tmin = 1
        while tmin <= dmax and cnt[tmin] > BLK:
            tmin += 1
        best = None
        for T in range(tmin, max(dmax, tmin) + 1):
            ms = [W] + [min(W, max(2, int(cnt[t]) + (int(cnt[t]) & 1)))
                        for t in range(1, T)]
            ov = np.maximum(dg - T, 0).sum(axis=1)
            V = int(np.ceil(ov / 128).max())
            cost = ((sum(ms) + 128 * V) * 1.06
                    + (T + V) * 25.0 + V * 110.0)
            if best is None or cost < best[0]:
                best = (cost, tuple(ms), V)
        ns.append(best[1])
        vb[g] = best[2]
    return assign, ns, vb


def _prepare_shards(node_attr, edge_attr, col):
    """Bucket edges: dense (first T_b per node, feature-major, nodes
    sorted by degree so tile t is trimmed to n_t columns) + overflow."""
    E = col.shape[0]
    NT = N_CORES * NODES_PER_CORE
    deg = np.bincount(col, minlength=NT)
    deg_gb = deg.reshape(TOTAL_BLOCKS, BLK)
    assign, ns, vb = _plan_layout(deg_gb)
    tb = np.array([len(n) for n in ns], np.int64)   # [NGRP]

    # node -> (core, group); group-wide slot = rank of the node by
    # descending degree over the core's whole 512-node group
    grp_of_gb = np.empty(TOTAL_BLOCKS, np.int64)
    core_of_gb = np.empty(TOTAL_BLOCKS, np.int64)
    loc = np.empty(NT, np.int64)               # node -> group slot 0..511
    node_perm = np.empty((N_CORES, NODES_PER_CORE), np.int64)
    W = GRP * BLK
    for g in range(NGRP):
        for c in range(N_CORES):
            gbs = [assign[g * GRP + q, c] for q in range(GRP)]
            for gb in gbs:
                grp_of_gb[gb] = g
                core_of_gb[gb] = c
            nodes = np.concatenate([gb * BLK + np.arange(BLK)
                                    for gb in gbs])
            degs = deg[nodes]
            order_ig = np.argsort(-degs, kind="stable")
            rows = nodes[order_ig]
            node_perm[c, g * W:(g + 1) * W] = rows
            loc[rows] = np.arange(W)

    order = np.argsort(col, kind="stable")
    col_s = col[order]
    starts = np.zeros(NT + 1, np.int64)
    starts[1:] = np.cumsum(deg)
    rank = np.arange(E, dtype=np.int64) - starts[col_s]
    gb_s = col_s >> 7
    grp_s = grp_of_gb[gb_s]
    core_s = core_of_gb[gb_s]
    T_s = tb[grp_s]                            # dense depth for each edge
    loc_s = loc[col_s]                         # group slot of dest node

    # dense tile-width prefix sums per group: nsum[g][r] = sum m_{g,<r}
    dense_elems = np.array([sum(n) for n in ns], np.int64)
    nsum = np.zeros((NGRP, int(tb.max()) + 1), np.int64)
    for g in range(NGRP):
        nsum[g, 1:tb[g] + 1] = np.cumsum(ns[g])

    # per-group per-partition elems and offsets (same for all cores)
    grp_elems = dense_elems + vb * 128         # [NGRP]
    grp_off = np.zeros(NGRP + 1, np.int64)
    grp_off[1:] = np.cumsum(grp_elems)
    tot_e = int(grp_off[-1])
    tot_v = int(vb.sum())
    v_off = np.zeros(NGRP + 1, np.int64)
    v_off[1:] = np.cumsum(vb)

    mask_d = rank < T_s
    mask_o = ~mask_d

    # overflow slot index within (core,group)
    okey = core_s[mask_o] * NGRP + grp_s[mask_o]
    oorder = np.argsort(okey, kind="stable")
    ocnt = np.bincount(okey, minlength=N_CORES * NGRP)
    ostarts = np.zeros(N_CORES * NGRP + 1, np.int64)
    ostarts[1:] = np.cumsum(ocnt)
    o_within = np.empty(len(okey), np.int64)
    o_within[oorder] = (np.arange(len(okey), dtype=np.int64)
                        - ostarts[okey[oorder]])

    vals16 = edge_attr.astype(np.float16)

    natp = np.zeros((NT, D), np.float32)
    natp[:NUM_NODES] = node_attr

    edges_by_core = []
    colv_by_core = []
    natT_by_core = []
    ar = np.arange(D)
    for c in range(N_CORES):
        earr = np.zeros((P, tot_e), np.float16)
        carr = np.full((P, max(tot_v, 1)), -1.0, np.float32)

        # dense: feature-major column at grp_off[g] + nsum[g,rank] + slot
        m = mask_d & (core_s == c)
        cidx = (grp_off[grp_s[m]] + nsum[grp_s[m], rank[m]] + loc_s[m])
        earr[:, cidx] = vals16[order[m]].T

        # overflow: slot-major; dest nodes all sit in group columns <128
        m2 = mask_o & (core_s == c)
        w = o_within[(core_s[mask_o] == c)]
        vt = w >> 7
        sl = w & 127
        base = (grp_off[grp_s[m2]] + dense_elems[grp_s[m2]] + vt * 128)
        earr[sl[:, None], (base[:, None] + ar[None, :])] = vals16[order[m2]]
        assert (loc_s[m2] < BLK).all()
        carr[sl, v_off[grp_s[m2]] + vt] = loc_s[m2].astype(np.float32)

        edges_by_core.append(earr)
        colv_by_core.append(carr)
        natT_by_core.append(np.ascontiguousarray(
            natp[node_perm[c]].T.astype(np.float16)))
    return (tuple(tuple(n) for n in ns), tuple(int(x) for x in vb),
            edges_by_core, colv_by_core, natT_by_core, node_perm)


_out_layout = {"trivial": False, "node_perm": None}


def assemble_core_out(arr):
    """[NGRP, 128, GRP*128] fp16 -> [12800, 128] f32 in position order.

    General path: feature-major out[g][f, q*128+j].
    Trivial-LN path: node-major out[g][j, q*128+f]."""
    a = np.asarray(arr).reshape(NGRP, P, GRP, BLK)
    if _out_layout["trivial"]:
        a = a.transpose(0, 2, 1, 3)
    else:
        a = a.transpose(0, 2, 3, 1)
    return a.reshape(NODES_PER_CORE, D).astype(np.float32)


def assemble_full(core_arrs):
    """Per-core raw out arrays -> full [NUM_NODES, D] f32 output."""
    perm = _out_layout["node_perm"]
    full = np.empty((N_CORES * NODES_PER_CORE, D), np.float32)
    for c in range(N_CORES):
        full[perm[c]] = assemble_core_out(core_arrs[c])
    return full[:NUM_NODES]


def kernel(node_attr, edge_attr, edge_index, W0, b0, W1, b1, W2, b2,
           ln_g, ln_b):
    from concourse import bass_utils

    node_attr = np.ascontiguousarray(np.asarray(node_attr, dtype=np.float32))
    edge_attr = np.ascontiguousarray(np.asarray(edge_attr, dtype=np.float32))
    col = np.asarray(edge_index)[1].astype(np.int64)
    W0 = np.asarray(W0, dtype=np.float64)
    W1 = np.asarray(W1, dtype=np.float64)
    W2 = np.asarray(W2, dtype=np.float64)
    b2v = np.asarray(b2, dtype=np.float64)
    # center W2 columns (per output feature) so LN mean vanishes
    W2c = W2 - W2.mean(axis=1, keepdims=True)
    b2c = b2v - b2v.mean()

    w0a = np.ascontiguousarray(W0[:128].astype(np.float16))
    w0b = np.ascontiguousarray(W0[128:].astype(np.float16))
    w1 = np.ascontiguousarray(W1.astype(np.float16))
    w2c = np.ascontiguousarray(W2c.astype(np.float16))
    b0v = np.asarray(b0, np.float32).reshape(128, 1).copy()
    b1v = np.asarray(b1, np.float32).reshape(128, 1).copy()
    b2cv = b2c.astype(np.float32).reshape(128, 1).copy()
    gam = np.asarray(ln_g, np.float32).reshape(128, 1).copy()
    bet = np.asarray(ln_b, np.float32).reshape(128, 1).copy()

    ns, vb, edges_by_core, colv_by_core, natT_by_core, node_perm = \
        _prepare_shards(node_attr, edge_attr, col)

    trivial = (np.allclose(np.asarray(ln_g), 1.0)
               and np.allclose(np.asarray(ln_b), 0.0))
    _out_layout["trivial"] = trivial
    _out_layout["node_perm"] = node_perm
    kb = ("v5t" if trivial else "v5", ns, vb)
    if kb not in _nc_cache:
        _nc_cache[kb] = _build_nc(kb)
    nc = _nc_cache[kb]

    iota_rep = np.ascontiguousarray(
        np.broadcast_to(np.arange(128, dtype=np.float16), (P, P)))
    shared = {"w0a": w0a, "w0b": w0b, "w1": w1, "w2c": w2c,
              "b0": b0v, "b1": b1v, "b2c": b2cv, "gam": gam, "bet": bet,
              "iota": iota_rep}
    in_maps = []
    for c in range(N_CORES):
        m = {"edges": edges_by_core[c], "colv": colv_by_core[c],
             "natT": natT_by_core[c]}
        m.update(shared)
        in_maps.append(m)

    res = bass_utils.run_bass_kernel_spmd(nc, in_maps,
                                          core_ids=list(range(N_CORES)))
    last_run_info["results"] = res
    last_run_info["nc"] = nc
    last_run_info["in_maps"] = in_maps
    last_run_info["kb"] = kb

    return assemble_full([res.results[c]["out"] for c in range(N_CORES)])


# revision 93
# speedup vs baseline: 1.0380x; 1.0380x over previous
"""Trainium2 Bass kernel for nn_NodeBlock (GNN message passing).

Pipeline: segment_sum of edge features onto destination nodes, concat with
node features, 3-layer MLP, LayerNorm.

Sharding: nodes are range-sharded across the 8 cores (12800 nodes/core, 100
blocks of 128). On the host, edges are bucketed by destination-node block.
Within a block, the first T edges of each node are packed "dense":
feature-major tiles dT[f, t*128 + j] = (t-th edge of node j)[f], so the
on-device aggregation for them is a plain PSUM accumulation via an
identity-weight matmul (no one-hot needed). Edges beyond T per node go to
"overflow" tiles in slot-major layout with a per-tile one-hot built by a
DVE is_equal against an iota row (like a classic scatter-add matmul).

All edge/node/weight data is fp16 (rel err ~3e-4, far inside the 2e-2
budget), so the PE runs at 1 cycle/row and edge HBM traffic is 2B/elem.

MLP runs feature-major, batched over groups of 4 blocks (free dim 512).
LayerNorm: W2 is column-centered on the host so the mean term vanishes;
variance comes from an ACT Square+accum_out on the transposed block;
rstd is folded into the PSUM evacuation; gamma/beta are folded into one
ACT (scale/bias per-partition) after transposing back to feature-major.
Output is fp16 feature-major; the host transposes/upcasts.
"""

import sys

sys.path.insert(0, "/opt/trn_rl_repo")

import numpy as np

N_CORES = 8
NUM_NODES = 100000
D = 128            # node/edge feature dim
P = 128            # partitions
BLK = 128          # nodes per block
BLOCKS_PER_CORE = 100
NODES_PER_CORE = BLK * BLOCKS_PER_CORE   # 12800
TOTAL_BLOCKS = N_CORES * BLOCKS_PER_CORE  # 800
GRP = 4            # blocks per MLP group (free dim 512)
NGRP = BLOCKS_PER_CORE // GRP            # 25
EPS = 1e-5

_nc_cache = {}
last_run_info = {}

TUNE = {
    "ovf_lambda": 0.3,  # extra planner cost per overflow (IS_EQ) tile
    "chunk": 2,         # blocks per edge DMA
    "ebufs": 8, "ohbufs": 16, "sbufs": 4,
    "agbufs": 3, "mlpbufs": 3, "tyxbufs": 2,
    "out_sync_tail": 5,   # last N groups' out-DMAs go to the sync queue
    "prefetch_pairs": 2,  # loop mode: chunks prefetched across the barrier
    "split_evac": True,
    "out_queue": "gpsimd",      # gpsimd | dma_queue setting
    "oh_engine": "vector",      # vector | gpsimd | split
    "aggr_evac": "vector",      # vector | scalar
    "relu_engine": "scalar",    # scalar | vector
    "stats_engine": "scalar",   # scalar (Square+accum) | vector | split
    "yot_engine": "scalar",     # scalar | gpsimd
    "xn_engine": "vector",      # vector | scalar
    "dma_queue": "sync",        # sync | alt  (stream DMA issue queue)
}


def _build_nc(kb, loop_iters=None):
    """kb: ("v3"|"v3t", (T_b,)*100, (V_b,)*100) — per-position dense depth
    T_b and overflow tile count V_b, shared across the 8 cores.
    "v3t" = trivial LayerNorm affine (gamma==1, beta==0): skip the
    gamma/beta stage and the second transpose; output is node-major."""
    import contextlib
    import concourse.bacc as bacc
    import concourse.tile as tile
    import concourse.mybir as mybir
    from concourse.masks import make_identity

    dt = mybir.dt
    f32 = dt.float32
    f16 = dt.float16
    variant, ns, vb = kb
    trivial = variant.endswith("t")
    ns = [list(n) for n in ns]           # per-GROUP dense tile widths
    vb = list(vb)
    tot_v = sum(vb)                      # total overflow tiles per core
    grp_elems = [sum(n) + v * 128 for n, v in zip(ns, vb)]
    tot_e = sum(grp_elems)
    # two chunks per group, split at a dense-tile boundary near half
    chunk_spans = []
    for g in range(NGRP):
        half = grp_elems[g] // 2
        acc = 0
        s0 = 0
        for m in ns[g]:
            if acc + m > half and acc > 0:
                break
            acc += m
        s0 = acc
        chunk_spans.append((s0, grp_elems[g] - s0))

    nc = bacc.Bacc("TRN2", target_bir_lowering=False, debug=False,
                   name="nodeblock")

    edges = nc.dram_tensor("edges", [P, tot_e], f16, kind="ExternalInput")
    iota_in = nc.dram_tensor("iota", [P, P], f16, kind="ExternalInput")
    colv = nc.dram_tensor("colv", [P, max(tot_v, 1)], f32,
                          kind="ExternalInput")
    natT = nc.dram_tensor("natT", [P, NODES_PER_CORE], f16,
                          kind="ExternalInput")
    w_in = {}
    for nm in ["w0a", "w0b", "w1", "w2c"]:
        w_in[nm] = nc.dram_tensor(nm, [128, 128], f16, kind="ExternalInput")
    for nm in ["b0", "b1", "b2c", "gam", "bet"]:
        w_in[nm] = nc.dram_tensor(nm, [128, 1], f32, kind="ExternalInput")
    out = nc.dram_tensor("out", [NGRP, P, GRP * BLK], f16,
                         kind="ExternalOutput")

    with tile.TileContext(nc) as tc:
        with (
            tc.tile_pool(name="const", bufs=1) as cpool,
            tc.tile_pool(name="edge", bufs=TUNE["ebufs"]) as epool,
            tc.tile_pool(name="oh", bufs=TUNE["ohbufs"]) as ohpool,
            tc.tile_pool(name="small", bufs=TUNE["sbufs"]) as spool,
            tc.tile_pool(name="psag", bufs=TUNE["agbufs"],
                         space="PSUM") as psag,
            tc.tile_pool(name="psmlp", bufs=TUNE["mlpbufs"],
                         space="PSUM") as psmlp,
            tc.tile_pool(name="pstyx", bufs=TUNE["tyxbufs"],
                         space="PSUM") as pstyx,
        ):
            # small/gating constants first: iota+colv gate every IS_EQ,
            # weights gate the first MLP; the 3.3MB natT load goes last
            iota = cpool.tile([P, P], f16, tag="iota", name="iota")
            nc.scalar.dma_start(out=iota[:], in_=iota_in[:])
            colv_s = cpool.tile([P, max(tot_v, 1)], f32, tag="colv",
                                name="colv")
            nc.scalar.dma_start(out=colv_s[:], in_=colv[:])
            consts = {}
            for nm, t in w_in.items():
                cdt = f16 if nm in ("w0a", "w0b", "w1", "w2c") else f32
                consts[nm] = cpool.tile(list(t.shape), cdt, tag=nm, name=nm)
                nc.scalar.dma_start(out=consts[nm][:], in_=t[:])
            natT_s = cpool.tile([P, NODES_PER_CORE], f16, tag="natT",
                                name="natT")
            nc.scalar.dma_start(out=natT_s[:], in_=natT[:])
            ident = cpool.tile([P, P], f16, tag="ident", name="ident")
            make_identity(nc, ident[:])
            consts["iota"] = iota
            epst = cpool.tile([P, 1], f32, tag="eps", name="eps")
            nc.vector.memset(epst[:], EPS)
            consts["eps"] = epst
            ones_c = cpool.tile([P, 1], f16, tag="ones_c", name="ones_c")
            nc.vector.memset(ones_c[:], 1.0)
            consts["ones_c"] = ones_c
            ones_r = cpool.tile([1, P], f32, tag="ones_r", name="ones_r")
            nc.vector.memset(ones_r[:], 1.0)
            consts["ones_r"] = ones_r

            # prefetch tiles for the first chunks: loaded in the preamble,
            # then re-loaded at the END of each loop-body execution so the
            # next iteration's aggregation starts with data already in SBUF
            # flat chunk list: (group, span); prefetch the first npf
            flat_chunks = []
            eo = 0
            for g in range(NGRP):
                for s in chunk_spans[g]:
                    flat_chunks.append((eo, s))
                    eo += s
            npf = TUNE.get("prefetch_pairs", 0)
            pf_tiles = []
            for i in range(npf):
                eoff, span = flat_chunks[i]
                pft = cpool.tile([P, span], f16, tag=f"pf{i}",
                                 name=f"pf{i}")
                nc.sync.dma_start(out=pft[:],
                                  in_=edges[:, eoff:eoff + span])
                pf_tiles.append((pft, eoff, span))

            loop_cm = (tc.For_i(0, loop_iters, 1) if loop_iters
                       else contextlib.nullcontext())
            with loop_cm:
                _emit_blocks(nc, tc, ns, vb, chunk_spans, epool, ohpool,
                             spool, psag, psmlp, pstyx, colv_s, natT_s,
                             consts, ident, edges, out, mybir, trivial,
                             pf_tiles)
                if loop_iters:
                    for pft, eoff, span in pf_tiles:
                        nc.sync.dma_start(out=pft[:],
                                          in_=edges[:, eoff:eoff + span])
    nc.finalize()
    return nc


def _emit_blocks(nc, tc, ns, vb, chunk_spans, epool, ohpool, spool, psag,
                 psmlp, pstyx, colv_s, natT_s, consts, ident, edges, out,
                 mybir, trivial=False, pf_tiles=()):
    dt = mybir.dt
    f32 = dt.float32
    f16 = dt.float16
    Alu = mybir.AluOpType
    Act = mybir.ActivationFunctionType
    iota = consts["iota"]
    W = GRP * BLK

    grp_elems = [sum(n) + v * 128 for n, v in zip(ns, vb)]
    max_span = max(max(s) for s in chunk_spans)
    g_offs, v_offs = [], []
    eo = vo = 0
    for g in range(NGRP):
        g_offs.append(eo)
        v_offs.append(vo)
        eo += grp_elems[g]
        vo += vb[g]

    for g in range(NGRP):
        # two chunk DMAs per group (split at a dense-tile boundary)
        s0, s1 = chunk_spans[g]
        tiles_c = []
        coff = g_offs[g]
        for ci, span in enumerate((s0, s1)):
            idx = 2 * g + ci
            if idx < len(pf_tiles):
                tiles_c.append(pf_tiles[idx][0])
            else:
                ct = epool.tile([P, max_span], f16, tag="eblk",
                                name="eblk")
                nc.sync.dma_start(out=ct[:, :span],
                                  in_=edges[:, coff:coff + span])
                tiles_c.append(ct)
            coff += span

        def ebslice(off, width):
            if off + width <= s0:
                return tiles_c[0][:, off:off + width]
            return tiles_c[1][:, off - s0:off - s0 + width]

        pag = psag.tile([P, W], f32, tag="ag", name="ag")
        ssq = spool.tile([P, GRP], f32, tag="ssq", name="ssq")
        py_tiles = []
        V = vb[g]
        # one-hots first so DVE runs ahead of the PE
        ohs = []
        for v in range(V):
            oh = ohpool.tile([P, 128], f16, tag="oh", name="oh")
            nc.vector.tensor_scalar(
                out=oh[:], in0=iota[:],
                scalar1=colv_s[:, v_offs[g] + v:v_offs[g] + v + 1],
                scalar2=None, op0=Alu.is_equal)
            ohs.append(oh)
        # single group-wide dense accumulation chain, then overflow
        N = ns[g]
        T = len(N)
        off = 0
        for t, n in enumerate(N):
            nc.tensor.matmul(out=pag[:, :n], lhsT=ident[:],
                             rhs=ebslice(off, n),
                             start=(t == 0),
                             stop=(t == T - 1 and V == 0),
                             skip_group_check=True)
            off += n
        for v in range(V):
            nc.tensor.matmul(
                out=pag[:, :BLK], lhsT=ebslice(off + v * 128, 128),
                rhs=ohs[v][:], start=False, stop=(v == V - 1),
                skip_group_check=True)

        # aggregated fp16 copy PSUM -> SBUF
        aggrT = spool.tile([P, GRP * BLK], f16, tag="aggrT", name="aggrT")
        H = GRP * BLK // 2
        if TUNE["split_evac"]:
            # two parallel half-evacs on different engines so the first
            # half of the W0b matmul can start sooner
            nc.vector.tensor_copy(aggrT[:, :H], pag[:, :H])
            nc.scalar.copy(aggrT[:, H:], pag[:, H:])
        elif TUNE["aggr_evac"] == "vector":
            nc.vector.tensor_copy(aggrT[:], pag[:])
        else:
            nc.scalar.copy(aggrT[:], pag[:])

        # MLP over the whole group (free dim 512)
        ph1 = psmlp.tile([P, GRP * BLK], f32, tag="mlp", name="mlp")
        nc.tensor.matmul(out=ph1[:], lhsT=consts["w0a"][:],
                         rhs=natT_s[:, g * GRP * BLK:(g + 1) * GRP * BLK],
                         start=True, stop=False)
        if TUNE["split_evac"]:
            nc.tensor.matmul(out=ph1[:, :H], lhsT=consts["w0b"][:],
                             rhs=aggrT[:, :H], start=False, stop=True,
                             skip_group_check=True)
            nc.tensor.matmul(out=ph1[:, H:], lhsT=consts["w0b"][:],
                             rhs=aggrT[:, H:], start=False, stop=True,
                             skip_group_check=True)
        else:
            nc.tensor.matmul(out=ph1[:], lhsT=consts["w0b"][:],
                             rhs=aggrT[:], start=False, stop=True)
        h1 = spool.tile([P, GRP * BLK], f16, tag="h1", name="h1")
        if TUNE["relu_engine"] == "scalar":
            nc.scalar.activation(h1[:], ph1[:], Act.Relu,
                                 bias=consts["b0"][:])
        else:
            nc.vector.tensor_scalar(out=h1[:], in0=ph1[:],
                                    scalar1=consts["b0"][:], scalar2=0.0,
                                    op0=Alu.add, op1=Alu.max)

        ph2 = psmlp.tile([P, GRP * BLK], f32, tag="mlp", name="mlp")
        nc.tensor.matmul(out=ph2[:], lhsT=consts["w1"][:], rhs=h1[:],
                         start=True, stop=True)
        h2 = spool.tile([P, GRP * BLK], f16, tag="h2", name="h2")
        nc.scalar.activation(h2[:], ph2[:], Act.Relu, bias=consts["b1"][:])

        ph3 = psmlp.tile([P, GRP * BLK], f32, tag="mlp", name="mlp")
        nc.tensor.matmul(out=ph3[:], lhsT=consts["w2c"][:], rhs=h2[:],
                         start=True, stop=True)
        h3T = spool.tile([P, GRP * BLK], f16, tag="h3T", name="h3T")
        nc.scalar.activation(h3T[:], ph3[:], Act.Identity,
                             bias=consts["b2c"][:])

        # per block: transpose to node-major, Square+accum for variance
        tyx_w = GRP * BLK if trivial else 2 * GRP * BLK
        tyx = pstyx.tile([P, tyx_w], f16, tag="tyx", name="tyx")
        py_g = tyx[:, :GRP * BLK]
        pxt = None if trivial else tyx[:, GRP * BLK:]
        for q in range(GRP):
            py = py_g[:, q * BLK:(q + 1) * BLK]
            nc.tensor.transpose(py, h3T[:, q * BLK:(q + 1) * BLK],
                                ident[:])
            py_tiles.append(py)
            se = TUNE["stats_engine"]
            if se == "split":
                se = "scalar" if q % 2 == 0 else "vector"
            if se == "scalar":
                sq = spool.tile([P, BLK], f16, tag="sq", name="sq")
                nc.scalar.activation(sq[:], py, Act.Square,
                                     accum_out=ssq[:, q:q + 1])
            else:
                sq = spool.tile([P, BLK], f16, tag="sq", name="sq")
                nc.vector.tensor_tensor_reduce(
                    out=sq[:], in0=py, in1=py, scale=1.0, scalar=0.0,
                    op0=Alu.mult, op1=Alu.add, accum_out=ssq[:, q:q + 1])

        # rstd for the 4 blocks at once: 1/sqrt(ssq/128 + eps)
        std = spool.tile([P, GRP], f32, tag="std", name="std")
        nc.scalar.activation(std[:], ssq[:], Act.Sqrt,
                             bias=consts["eps"][:], scale=1.0 / BLK)
        rstd = spool.tile([P, GRP], f32, tag="rstd", name="rstd")
        nc.vector.reciprocal(rstd[:], std[:])

        # xn = py * rstd (node-major)
        if trivial:
            # gamma==1, beta==0: xn IS the output (node-major layout)
            xn_g = spool.tile([P, GRP * BLK], f16, tag="yoT", name="yoT")
            for q in range(GRP):
                xn = xn_g[:, q * BLK:(q + 1) * BLK]
                if TUNE["xn_engine"] == "vector":
                    nc.vector.tensor_scalar(out=xn, in0=py_tiles[q],
                                            scalar1=rstd[:, q:q + 1],
                                            scalar2=None, op0=Alu.mult)
                else:
                    nc.scalar.activation(xn, py_tiles[q], Act.Copy,
                                         scale=rstd[:, q:q + 1])
            yoT = xn_g
            if g >= NGRP - TUNE.get("out_sync_tail", 0):
                nc.sync.dma_start(out=out[g], in_=yoT[:])
                continue
        else:
            for q in range(GRP):
                xn = spool.tile([P, BLK], f16, tag="xn", name="xn")
                if TUNE["xn_engine"] == "vector":
                    nc.vector.tensor_scalar(out=xn[:], in0=py_tiles[q],
                                            scalar1=rstd[:, q:q + 1],
                                            scalar2=None, op0=Alu.mult)
                else:
                    nc.scalar.activation(xn[:], py_tiles[q], Act.Copy,
                                         scale=rstd[:, q:q + 1])
                nc.tensor.transpose(pxt[:, q * BLK:(q + 1) * BLK], xn[:],
                                    ident[:])
            yoT = spool.tile([P, GRP * BLK], f16, tag="yoT", name="yoT")
            if TUNE["yot_engine"] == "scalar":
                nc.scalar.activation(yoT[:], pxt[:], Act.Identity,
                                     bias=consts["bet"][:],
                                     scale=consts["gam"][:])
            else:
                nc.gpsimd.tensor_scalar(out=yoT[:], in0=pxt[:],
                                        scalar1=consts["gam"][:],
                                        scalar2=consts["bet"][:],
                                        op0=Alu.mult, op1=Alu.add)
        if TUNE["out_queue"] == "gpsimd":
            odma = nc.gpsimd
        elif TUNE["dma_queue"] == "sync":
            odma = nc.sync
        else:
            odma = nc.scalar if g % 2 == 0 else nc.sync
        odma.dma_start(out=out[g], in_=yoT[:])


def _plan_layout(deg_gb):
    """deg_gb: [800, 128] per-global-block node degrees.

    Assign the 800 blocks to 100 positions x 8 cores so that blocks with
    similar degree profiles share a position (minimizing the cross-core
    max padding), and pick a per-position dense depth T_b minimizing
    tiles = T_b + max_core ceil(overflow/128). With nodes sorted by
    degree within each block, dense tile t only needs
    n_t = max_core #{deg > t} columns; returns those widths too."""
    order = np.argsort(-deg_gb.sum(axis=1), kind="stable")
    assign = order.reshape(BLOCKS_PER_CORE, N_CORES)  # [pos, core] -> gb
    W = GRP * BLK
    ns = []
    vb = np.zeros(NGRP, np.int64)
    for g in range(NGRP):
        # per-core degree profile over the whole 512-node group
        dg = np.concatenate(
            [deg_gb[assign[g * GRP + q]] for q in range(GRP)],
            axis=1)                            # [8, 512]
        dmax = int(dg.max())
        cnt = np.stack([(dg > t).sum(axis=1).max(axis=0)
                        for t in range(max(dmax, 1) + 1)])  # max_c count
        # overflow nodes must fit the first 128 group columns
        tmin = 1
        while tmin <= dmax and cnt[tmin] > BLK:
            tmin += 1
        best = None
        for T in range(tmin, max(dmax, tmin) + 1):
            ms = [W] + [min(W, max(2, int(cnt[t]) + (int(cnt[t]) & 1)))
                        for t in range(1, T)]
            ov = np.maximum(dg - T, 0).sum(axis=1)
            V = int(np.ceil(ov / 128).max())
            cost = ((sum(ms) + 128 * V) * 1.06
                    + (T + V) * 25.0 + V * 110.0)
            if best is None or cost < best[0]:
                best = (cost, tuple(ms), V)
        ns.append(best[1])
        vb[g] = best[2]
    return assign, ns, vb


def _prepare_shards(node_attr, edge_attr, col):
    """Bucket edges: dense (first T_b per node, feature-major, nodes
    sorted by degree so tile t is trimmed to n_t columns) + overflow."""
    E = col.shape[0]
    NT = N_CORES * NODES_PER_CORE
    deg = np.bincount(col, minlength=NT)
    deg_gb = deg.reshape(TOTAL_BLOCKS, BLK)
    assign, ns, vb = _plan_layout(deg_gb)
    tb = np.array([len(n) for n in ns], np.int64)   # [NGRP]

    # node -> (core, group); group-wide slot = rank of the node by
    # descending degree over the core's whole 512-node group
    grp_of_gb = np.empty(TOTAL_BLOCKS, np.int64)
    core_of_gb = np.empty(TOTAL_BLOCKS, np.int64)
    loc = np.empty(NT, np.int64)               # node -> group slot 0..511
    node_perm = np.empty((N_CORES, NODES_PER_CORE), np.int64)
    W = GRP * BLK
    for g in range(NGRP):
        for c in range(N_CORES):
            gbs = [assign[g * GRP + q, c] for q in range(GRP)]
            for gb in gbs:
                grp_of_gb[gb] = g
                core_of_gb[gb] = c
            nodes = np.concatenate([gb * BLK + np.arange(BLK)
                                    for gb in gbs])
            degs = deg[nodes]
            order_ig = np.argsort(-degs, kind="stable")
            rows = nodes[order_ig]
            node_perm[c, g * W:(g + 1) * W] = rows
            loc[rows] = np.arange(W)

    order = np.argsort(col, kind="stable")
    col_s = col[order]
    starts = np.zeros(NT + 1, np.int64)
    starts[1:] = np.cumsum(deg)
    rank = np.arange(E, dtype=np.int64) - starts[col_s]
    gb_s = col_s >> 7
    grp_s = grp_of_gb[gb_s]
    core_s = core_of_gb[gb_s]
    T_s = tb[grp_s]                            # dense depth for each edge
    loc_s = loc[col_s]                         # group slot of dest node

    # dense tile-width prefix sums per group: nsum[g][r] = sum m_{g,<r}
    dense_elems = np.array([sum(n) for n in ns], np.int64)
    nsum = np.zeros((NGRP, int(tb.max()) + 1), np.int64)
    for g in range(NGRP):
        nsum[g, 1:tb[g] + 1] = np.cumsum(ns[g])

    # per-group per-partition elems and offsets (same for all cores)
    grp_elems = dense_elems + vb * 128         # [NGRP]
    grp_off = np.zeros(NGRP + 1, np.int64)
    grp_off[1:] = np.cumsum(grp_elems)
    tot_e = int(grp_off[-1])
    tot_v = int(vb.sum())
    v_off = np.zeros(NGRP + 1, np.int64)
    v_off[1:] = np.cumsum(vb)

    mask_d = rank < T_s
    mask_o = ~mask_d

    # overflow slot index within (core,group)
    okey = core_s[mask_o] * NGRP + grp_s[mask_o]
    oorder = np.argsort(okey, kind="stable")
    ocnt = np.bincount(okey, minlength=N_CORES * NGRP)
    ostarts = np.zeros(N_CORES * NGRP + 1, np.int64)
    ostarts[1:] = np.cumsum(ocnt)
    o_within = np.empty(len(okey), np.int64)
    o_within[oorder] = (np.arange(len(okey), dtype=np.int64)
                        - ostarts[okey[oorder]])

    vals16 = edge_attr.astype(np.float16)

    natp = np.zeros((NT, D), np.float32)
    natp[:NUM_NODES] = node_attr

    edges_by_core = []
    colv_by_core = []
    natT_by_core = []
    ar = np.arange(D)
    for c in range(N_CORES):
        earr = np.zeros((P, tot_e), np.float16)
        carr = np.full((P, max(tot_v, 1)), -1.0, np.float32)

        # dense: feature-major column at grp_off[g] + nsum[g,rank] + slot
        m = mask_d & (core_s == c)
        cidx = (grp_off[grp_s[m]] + nsum[grp_s[m], rank[m]] + loc_s[m])
        earr[:, cidx] = vals16[order[m]].T

        # overflow: slot-major; dest nodes all sit in group columns <128
        m2 = mask_o & (core_s == c)
        w = o_within[(core_s[mask_o] == c)]
        vt = w >> 7
        sl = w & 127
        base = (grp_off[grp_s[m2]] + dense_elems[grp_s[m2]] + vt * 128)
        earr[sl[:, None], (base[:, None] + ar[None, :])] = vals16[order[m2]]
        assert (loc_s[m2] < BLK).all()
        carr[sl, v_off[grp_s[m2]] + vt] = loc_s[m2].astype(np.float32)

        edges_by_core.append(earr)
        colv_by_core.append(carr)
        natT_by_core.append(np.ascontiguousarray(
            natp[node_perm[c]].T.astype(np.float16)))
    return (tuple(tuple(n) for n in ns), tuple(int(x) for x in vb),
            edges_by_core, colv_by_core, natT_by_core, node_perm)


_out_layout = {"trivial": False, "node_perm": None}


def assemble_core_out(arr):
    """[NGRP, 128, GRP*128] fp16 -> [12800, 128] f32 in position order.

    General path: feature-major out[g][f, q*128+j].
    Trivial-LN path: node-major out[g][j, q*128+f]."""
    a = np.asarray(arr).reshape(NGRP, P, GRP, BLK)
    if _out_layout["trivial"]:
        a = a.transpose(0, 2, 1, 3)
    else:
        a = a.transpose(0, 2, 3, 1)
    return a.reshape(NODES_PER_CORE, D).astype(np.float32)


def assemble_full(core_arrs):
    """Per-core raw out arrays -> full [NUM_NODES, D] f32 output."""
    perm = _out_layout["node_perm"]
    full = np.empty((N_CORES * NODES_PER_CORE, D), np.float32)
    for c in range(N_CORES):
        full[perm[c]] = assemble_core_out(core_arrs[c])
    return full[:NUM_NODES]


def kernel(node_attr, edge_attr, edge_index, W0, b0, W1, b1, W2, b2,
           ln_g, ln_b):
    from concourse import bass_utils

    node_attr = np.ascontiguousarray(np.asarray(node_attr, dtype=np.float32))
    edge_attr = np.ascontiguousarray(np.asarray(edge_attr, dtype=np.float32))
    col = np.asarray(edge_index)[1].astype(np.int64)
    W0 = np.asarray(W0, dtype=np.float64)
    W1 = np.asarray(W1, dtype=np.float64)
    W2 = np.asarray(W2, dtype=np.float64)
    b2v = np.asarray(b2, dtype=np.float64)
    # center W2 columns (per output feature) so LN mean vanishes
    W2c = W2 - W2.mean(axis=1, keepdims=True)
    b2c = b2v - b2v.mean()

    w0a = np.ascontiguousarray(W0[:128].astype(np.float16))
    w0b = np.ascontiguousarray(W0[128:].astype(np.float16))
    w1 = np.ascontiguousarray(W1.astype(np.float16))
    w2c = np.ascontiguousarray(W2c.astype(np.float16))
    b0v = np.asarray(b0, np.float32).reshape(128, 1).copy()
    b1v = np.asarray(b1, np.float32).reshape(128, 1).copy()
    b2cv = b2c.astype(np.float32).reshape(128, 1).copy()
    gam = np.asarray(ln_g, np.float32).reshape(128, 1).copy()
    bet = np.asarray(ln_b, np.float32).reshape(128, 1).copy()

    ns, vb, edges_by_core, colv_by_core, natT_by_core, node_perm = \
        _prepare_shards(node_attr, edge_attr, col)

    trivial = (np.allclose(np.asarray(ln_g), 1.0)
               and np.allclose(np.asarray(ln_b), 0.0))
    _out_layout["trivial"] = trivial
    _out_layout["node_perm"] = node_perm
    kb = ("v5t" if trivial else "v5", ns, vb)
    if kb not in _nc_cache:
        _nc_cache[kb] = _build_nc(kb)
    nc = _nc_cache[kb]

    iota_rep = np.ascontiguousarray(
        np.broadcast_to(np.arange(128, dtype=np.float16), (P, P)))
    shared = {"w0a": w0a, "w0b": w0b, "w1": w1, "w2c": w2c,
              "b0": b0v, "b1": b1v, "b2c": b2cv, "gam": gam, "bet": bet,
              "iota": iota_rep}
    in_maps = []
    for c in range(N_CORES):
        m = {"edges": edges_by_core[c], "colv": colv_by_core[c],
             "natT": natT_by_core[c]}
        m.update(shared)
        in_maps.append(m)

    res = bass_utils.run_bass_kernel_spmd(nc, in_maps,
                                          core_ids=list(range(N_CORES)))
    last_run_info["results"] = res
    last_run_info["nc"] = nc
    last_run_info["in_maps"] = in_maps
    last_run_info["kb"] = kb

    return assemble_full([res.results[c]["out"] for c in range(N_CORES)])
